# revision 42
# baseline (speedup 1.0000x reference)
"""Trainium2 Bass kernel for nn_Drnet (histogram-binned multi-head MLP).

Contract: kernel(**inputs) takes the FULL unsharded inputs (t [N], x [N,100],
trunk + 5-head weights) and returns the FULL [N, 1] float32 output.

Strategy:
  * Host: compute each row's treatment bin exactly as the reference
    (floor(t*5) in fp32, clipped), stable-sort rows by bin, shard the sorted
    rows contiguously across 8 NeuronCores, and pad each per-core bin segment
    to a multiple of the 512-row tile so every device tile is single-bin.
    The per-tile head weights are gathered host-side into an input array, so
    one SPMD program serves all cores/tiles with weights arriving as data.
  * Layout: features live on SBUF partitions, rows on the free dim (x is
    pre-transposed host-side), so no on-device transposes are needed. The
    treatment t rides along as channel 96 of the input and is propagated
    through every layer via an extra unit weight column, which lets each
    Treat_Linear layer (feat@W + t*tw + b) be a single 65xK matmul.
  * Device per tile: 5 matmuls (trunk L1/L2, head L1/L2/L3) in bf16 with
    fp32 PSUM accumulation; relu+bias fused into the PSUM->SBUF evacuation,
    alternating between the Scalar and Vector engines to balance load.
    Head L3 outputs of 4 consecutive tiles land in one PSUM bank at
    partitions 0/32/64/96 so they are evacuated by a single activation op.
"""
import numpy as np
import ml_dtypes

import concourse.bass as bass
import concourse.tile as tile
from concourse import mybir
from concourse.bass_utils import run_bass_kernel_spmd
from concourse.vector_clock import ScopedClock
from contextlib import ExitStack

BF16 = ml_dtypes.bfloat16

NCORES = 8
N = 1_000_000
D = 100
H = 64
NH = 5
TILE = 512
RPC = N // NCORES            # 125000 rows per core
NT = 252                     # tiles per core (>= ceil((RPC + 5*511)/512), mult of 4)
RPAD = NT * TILE             # 129024 padded rows per core
NG = NT // 4                 # groups of 4 tiles sharing one output PSUM bank
KX = 101                     # input channels: 96 features, t, 4 features
TROW = 96                    # partition carrying t
HA = H + 1                   # augmented hidden dim (t channel at row 64)

_FP32 = mybir.dt.float32
_BF16 = mybir.dt.bfloat16


_MAX_WAITS = 1
_MAX_WAITS_BY_TYPE = {}
_DEFAULT_MAX_WAITS = 1


class _SplitDrainTileContext(tile.TileContext):
    """Workaround: this walrus build rejects >2 embedded sync waits per
    instruction. Excess waits are moved onto same-engine nops inserted
    immediately before the overloaded instruction (same semantics: the
    engine's sequencer satisfies them in program order). The kernel-tail
    Drain additionally gets its waits via a chain of SP nops."""

    def _split_excess_waits(self):
        nc = self.nc
        for f in nc.m.functions:
            for bb in f.blocks:
                new_list = []
                changed = False
                for inst in bb.instructions:
                    si = inst.sync_info
                    waits = list(si.on_wait) if si and si.on_wait else []
                    maxw = _MAX_WAITS_BY_TYPE.get(
                        type(inst).__name__, _DEFAULT_MAX_WAITS)
                    if len(waits) > maxw:
                        changed = True
                        excess, keep = waits[:-maxw], waits[-maxw:]
                        for i in range(0, len(excess), _DEFAULT_MAX_WAITS):
                            nop = mybir.InstNoOp(
                                name=nc.get_next_instruction_name(),
                                ins=[], outs=[])
                            nop.engine = inst.engine
                            nop.sync_info = mybir.SyncInfo(
                                on_wait=list(excess[i:i + _DEFAULT_MAX_WAITS]),
                                on_update=[])
                            nc.register_instruction(nop)
                            new_list.append(nop)
                        inst.sync_info = mybir.SyncInfo(
                            on_wait=keep,
                            on_update=list(si.on_update) if si.on_update else [])
                    new_list.append(inst)
                if changed:
                    bb.instructions[:] = new_list

    def _drain_and_barrier(self, tick_clock, wait_clock):
        gc = tick_clock.global_clock
        needs = []
        for scope, vc in ScopedClock({None: gc}).items():
            for proc in range(len(vc)):
                t = vc[proc]
                if t > 0:
                    needs.append((scope, proc, t))
        for scope, proc, t in needs:
            nop = self.nc.sync.nop()
            partial = ScopedClock()
            partial.require_at_least(scope, proc, t)
            wait_clock.add_sem_waits(nop.ins, partial)
        self.nc.sync.drain()
        self.nc.all_engine_barrier()
        assert self.sems is not None
        popped = self.nc._tile_sem_poison_stack.pop()
        assert popped is self._sem_poison
        self.nc.clear_and_free_semaphores(list(self.sems.allocated().values()))
        self.nc.all_engine_barrier()
        self._split_excess_waits()


NP = NT // 2                 # tile pairs per core
GT = 4 * TILE                # rows per group (4 tiles)

# group weight tile [128, WG] (bf16), one DMA per 4-tile group:
#   cols 130*pi : 130*pi+130   hw1|hw2|hw3 blocks for pair pi
#                              (even tile at rows 0:64, odd at 64:128)
#   row 96, cols 260+129*i ... htw1|htw2|htw3 for group-tile i (0..3)
#   col 776+2*pi = hb1 pair vector, col 777+2*pi = hb2 pair vector
#   col 780 rows {0,32,64,96} = hb3 of group-tile i
WBC = 130
HTB = 260                    # htw column base
BIAS0 = 776
WG = 784


def _build_program():
    nc = bass.Bass()
    xt_h = nc.dram_tensor("xt", [NG, KX, GT], _BF16, kind="ExternalInput")
    wg_h = nc.dram_tensor("wg", [NG, 128, WG], _BF16, kind="ExternalInput")
    bb_h = nc.dram_tensor("bb", [NG, 128, 3], _FP32, kind="ExternalInput")
    w1_h = nc.dram_tensor("w1", [2, KX, H], _BF16, kind="ExternalInput")
    w2_h = nc.dram_tensor("w2", [128, H], _BF16, kind="ExternalInput")
    b12_h = nc.dram_tensor("b12", [128, 2], _FP32, kind="ExternalInput")
    out_h = nc.dram_tensor("out", [NT, TILE], _FP32, kind="ExternalOutput")

    RELU = mybir.ActivationFunctionType.Relu
    IDENT = mybir.ActivationFunctionType.Identity
    ADD = mybir.AluOpType.add
    MAX = mybir.AluOpType.max

    with _SplitDrainTileContext(nc) as tc, ExitStack() as ctx:
        statics = ctx.enter_context(tc.tile_pool(name="statics", bufs=1))
        xpool = ctx.enter_context(tc.tile_pool(name="x", bufs=6))
        wpool = ctx.enter_context(tc.tile_pool(name="w", bufs=6))
        bpool = ctx.enter_context(tc.tile_pool(name="b", bufs=6))
        hpool = ctx.enter_context(tc.tile_pool(name="h", bufs=3))
        opool = ctx.enter_context(tc.tile_pool(name="o", bufs=2))
        pspool = ctx.enter_context(tc.tile_pool(name="ps", bufs=6, space="PSUM"))
        ps5pool = ctx.enter_context(tc.tile_pool(name="ps5", bufs=2, space="PSUM"))

        w1e_sb = statics.tile([KX, H], _BF16)
        nc.sync.dma_start(out=w1e_sb, in_=w1_h[0, :, :])
        w1o_sb = statics.tile([KX, H], _BF16)
        nc.sync.dma_start(out=w1o_sb, in_=w1_h[1, :, :])
        w2_sb = statics.tile([128, H], _BF16)
        nc.sync.dma_start(out=w2_sb, in_=w2_h[:, :])
        b12_sb = statics.tile([128, 2], _FP32)
        nc.sync.dma_start(out=b12_sb, in_=b12_h[:, :])

        # --- software-pipelined emission: stage k of pair p emits at
        # virtual step v = p + k, so each matmul's producer evacuation runs
        # ~2 steps (several PE ops) earlier and the PE never stalls. ---
        ST = {}   # pair -> state
        GS = {}   # group -> state

        def pair_geom(p):
            g, pi = divmod(p, 2)
            return {
                "g": g, "pi": pi,
                "c5": 64 * pi, "wc": WBC * pi,
                "hA": HTB + 129 * (2 * pi), "hB": HTB + 129 * (2 * pi + 1),
                "oA": GT // 2 * pi, "oB": GT // 2 * pi + TILE,
            }

        def LOAD(p):
            ST[p] = pair_geom(p)
            g, pi = ST[p]["g"], ST[p]["pi"]
            if pi == 0:
                xg = xpool.tile([KX, GT], _BF16, tag="xg")
                nc.sync.dma_start(out=xg, in_=xt_h[g, :, :])
                wt = wpool.tile([128, WG], _BF16, tag="wt")
                nc.sync.dma_start(out=wt, in_=wg_h[g, :, :])
                bb = bpool.tile([128, 3], _FP32, tag="bb")
                nc.sync.dma_start(out=bb, in_=bb_h[g, :, :])
                GS[g] = {"xg": xg, "wt": wt, "bb": bb}
            s = ST[p]
            s.update(GS[g])
            s["xA"] = s["xg"][:, s["oA"]:s["oA"] + TILE]
            s["xB"] = s["xg"][:, s["oB"]:s["oB"] + TILE]

        def MM1(p):
            s = ST[p]
            ps1 = pspool.tile([128, TILE], _FP32, tag="ps")
            nc.tensor.matmul(ps1[0:H, :], w1e_sb, s["xA"],
                             start=True, stop=True, tile_position=(0, 0))
            nc.tensor.matmul(ps1[H:128, :], w1o_sb, s["xB"],
                             start=True, stop=True, tile_position=(0, 64))
            s["ps1"] = ps1

        def ACT1(p):
            s = ST[p]
            h1 = hpool.tile([128, TILE], _BF16, tag="h1")
            nc.scalar.activation(h1, s["ps1"], RELU, bias=b12_sb[:, 0:1])
            s["h1"] = h1

        def MM2(p):
            s = ST[p]
            ps2 = pspool.tile([128, TILE], _FP32, tag="ps")
            h1 = s["h1"]
            nc.tensor.matmul(ps2[0:H, :], w2_sb[0:H, :], h1[0:H, :],
                             start=True, stop=True, tile_position=(0, 0))
            nc.tensor.matmul(ps2[H:128, :], w2_sb[H:128, :], h1[H:128, :],
                             start=True, stop=True, tile_position=(64, 64))
            s["ps2"] = ps2

        def ACT2(p):
            s = ST[p]
            h2 = hpool.tile([128, TILE], _BF16, tag="h2")
            nc.vector.tensor_scalar(
                out=h2, in0=s["ps2"], scalar1=b12_sb[:, 1:2], scalar2=0.0,
                op0=ADD, op1=MAX)
            s["h2"] = h2

        def MM3(p):
            s = ST[p]
            wt, h2, wc = s["wt"], s["h2"], s["wc"]
            ps3 = pspool.tile([128, TILE], _FP32, tag="ps")
            nc.tensor.matmul(ps3[0:H, :], wt[0:H, wc:wc + H], h2[0:H, :],
                             start=True, stop=False, tile_position=(0, 0))
            nc.tensor.matmul(ps3[0:H, :], wt[96:97, s["hA"]:s["hA"] + H],
                             s["xA"][96:97, :],
                             start=False, stop=True, tile_position=(96, 0))
            nc.tensor.matmul(ps3[H:128, :], wt[H:128, wc:wc + H], h2[H:128, :],
                             start=True, stop=False, tile_position=(64, 64))
            nc.tensor.matmul(ps3[H:128, :], wt[32:33, s["hB"]:s["hB"] + H],
                             s["xB"][32:33, :],
                             start=False, stop=True, tile_position=(32, 64))
            s["ps3"] = ps3

        def ACT3(p):
            s = ST[p]
            pi = s["pi"]
            a1 = hpool.tile([128, TILE], _BF16, tag="a1")
            nc.scalar.activation(
                a1, s["ps3"], RELU,
                bias=s["wt"][:, BIAS0 + 2 * pi:BIAS0 + 2 * pi + 1])
            s["a1"] = a1

        def MM4(p):
            s = ST[p]
            wt, a1, wc = s["wt"], s["a1"], s["wc"]
            ps4 = pspool.tile([128, TILE], _FP32, tag="ps")
            nc.tensor.matmul(ps4[0:H, :], wt[0:H, wc + H:wc + 2 * H],
                             a1[0:H, :],
                             start=True, stop=False, tile_position=(0, 0))
            nc.tensor.matmul(ps4[0:H, :],
                             wt[96:97, s["hA"] + H:s["hA"] + 2 * H],
                             s["xA"][96:97, :],
                             start=False, stop=True, tile_position=(96, 0))
            nc.tensor.matmul(ps4[H:128, :], wt[H:128, wc + H:wc + 2 * H],
                             a1[H:128, :],
                             start=True, stop=False, tile_position=(64, 64))
            nc.tensor.matmul(ps4[H:128, :],
                             wt[32:33, s["hB"] + H:s["hB"] + 2 * H],
                             s["xB"][32:33, :],
                             start=False, stop=True, tile_position=(32, 64))
            s["ps4"] = ps4

        def ACT4(p):
            s = ST[p]
            a2 = hpool.tile([128, TILE], _BF16, tag="a2")
            nc.vector.tensor_scalar(
                out=a2, in0=s["ps4"], scalar1=s["bb"][:, s["pi"]:s["pi"] + 1],
                scalar2=0.0, op0=ADD, op1=MAX)
            s["a2"] = a2

        def MM5(p):
            s = ST[p]
            g, pi, c5, wc = s["g"], s["pi"], s["c5"], s["wc"]
            wt, a2 = s["wt"], s["a2"]
            if pi == 0:
                GS[g]["ps5"] = ps5pool.tile([97, TILE], _FP32, tag="ps5",
                                            name=f"ps5_{g}")
            ps5 = GS[g]["ps5"]
            nc.tensor.matmul(ps5[c5:c5 + 1, :], wt[0:H, wc + 2 * H:wc + 2 * H + 1],
                             a2[0:H, :],
                             start=True, stop=False, tile_position=(0, c5))
            nc.tensor.matmul(ps5[c5:c5 + 1, :],
                             wt[96:97, s["hA"] + 2 * H:s["hA"] + 2 * H + 1],
                             s["xA"][96:97, :],
                             start=False, stop=True, tile_position=(96, c5))
            nc.tensor.matmul(ps5[c5 + 32:c5 + 33, :],
                             wt[H:128, wc + 2 * H:wc + 2 * H + 1],
                             a2[H:128, :],
                             start=True, stop=False,
                             tile_position=(64, c5 + 32))
            nc.tensor.matmul(ps5[c5 + 32:c5 + 33, :],
                             wt[32:33, s["hB"] + 2 * H:s["hB"] + 2 * H + 1],
                             s["xB"][32:33, :],
                             start=False, stop=True,
                             tile_position=(32, c5 + 32))

        def OB(p):
            s = ST.pop(p)
            if s["pi"] != 1:
                return
            g = s["g"]
            gs = GS.pop(g)
            ob = opool.tile([97, TILE], _FP32, tag="ob")
            if g % 2 == 0:
                nc.scalar.activation(ob, gs["ps5"], IDENT,
                                     bias=gs["bb"][0:97, 2:3])
            else:
                nc.vector.tensor_scalar(
                    out=ob, in0=gs["ps5"], scalar1=gs["bb"][0:97, 2:3],
                    scalar2=None, op0=ADD)
            nc.sync.dma_start(out=out_h[4 * g:4 * g + 4, :],
                              in_=ob[0:97:32, :])

        STAGES = [LOAD, None, MM1, ACT1, MM2, ACT2, MM3, ACT3,
                  MM4, ACT4, MM5, OB]
        for v in range(NP + len(STAGES) - 1):
            for k, stage in enumerate(STAGES):
                if stage is None:
                    continue
                p = v - k
                if 0 <= p < NP:
                    stage(p)
    return nc


_PROGRAM = None
last_results = None


def _get_program():
    global _PROGRAM
    if _PROGRAM is None:
        _PROGRAM = _build_program()
    return _PROGRAM


def make_in_maps(t, x, dW1, db1, dW2, db2,
                 hw1, htw1, hb1, hw2, htw2, hb2, hw3, htw3, hb3):
    """Host-side sharding/packing. Returns (in_maps, lidx_all, order)."""
    t = np.asarray(t, np.float32)
    x = np.asarray(x, np.float32)

    # --- bin + stable sort (binning identical to the reference) ---
    bins = np.clip(np.floor(t * np.float32(NH)).astype(np.int32), 0, NH - 1)
    order = np.argsort(bins, kind="stable")
    t_s = t[order]
    x_s = x[order]
    bins_s = bins[order]

    # --- trunk weights; t rides at input row 96 (even tiles) / 32 (odd) ---
    w1a = np.zeros((2, KX, H), np.float32)
    w1a[0, 0:96] = dW1[0:96]
    w1a[0, 97:KX] = dW1[96:D]
    w1a[1, 0:32] = dW1[0:32]
    w1a[1, 33:KX] = dW1[32:D]
    w2a = np.concatenate([dW2, dW2], axis=0).astype(np.float32)  # [128, H]
    b12 = np.zeros((128, 2), np.float32)
    b12[0:H, 0] = db1
    b12[H:128, 0] = db1
    b12[0:H, 1] = db2
    b12[H:128, 1] = db2

    # --- per-bin head weight blocks ---
    WQ = np.zeros((NH, H, WBC), np.float32)     # hw1 | hw2 | hw3
    TQ = np.zeros((NH, 1, 129), np.float32)     # htw1 | htw2 | htw3
    for q in range(NH):
        WQ[q, :, 0:H] = hw1[q]
        WQ[q, :, H:2 * H] = hw2[q]
        WQ[q, :, 2 * H] = hw3[q][:, 0]
        TQ[q, 0, 0:H] = htw1[q]
        TQ[q, 0, H:2 * H] = htw2[q]
        TQ[q, 0, 2 * H] = htw3[q, 0]
    HB1 = np.asarray(hb1, np.float32)
    HB2 = np.asarray(hb2, np.float32)
    hb3v = np.asarray(hb3, np.float32)[:, 0]

    # --- per-core padded tiling (each 512-row tile single-bin) ---
    in_maps = []
    lidx_all = []
    for c in range(NCORES):
        s = c * RPC
        tb = bins_s[s:s + RPC]
        parts = []
        tile_bins = []
        for q in range(NH):
            sel = np.nonzero(tb == q)[0].astype(np.int64)
            if len(sel) == 0:
                continue
            npad = (-len(sel)) % TILE
            parts.append(np.concatenate([sel, np.full(npad, -1, np.int64)]))
            tile_bins += [q] * ((len(sel) + npad) // TILE)
        lidx = np.concatenate(parts)
        rem = RPAD - len(lidx)
        assert rem >= 0 and rem % TILE == 0
        lidx = np.concatenate([lidx, np.full(rem, -1, np.int64)])
        tile_bins += [0] * (rem // TILE)
        tile_bins = np.asarray(tile_bins, np.int64)
        lidx_all.append(lidx)

        # augmented input [RPAD, KX]; even tiles carry t at ch 96, odd at
        # ch 32 (keeps the rank-1 t matmuls off the odd halves' array rows)
        safe = np.where(lidx >= 0, lidx, 0)
        xc = x_s[s:s + RPC]
        tc_ = t_s[s:s + RPC]
        feat = xc[safe]
        tval = tc_[safe]
        feat[lidx < 0] = 0.0
        tval[lidx < 0] = 0.0
        f3 = feat.reshape(NT, TILE, D)
        t3 = tval.reshape(NT, TILE)
        xa3 = np.empty((NT, TILE, KX), np.float32)
        xa3[0::2, :, 0:96] = f3[0::2, :, 0:96]
        xa3[0::2, :, 96] = t3[0::2]
        xa3[0::2, :, 97:KX] = f3[0::2, :, 96:D]
        xa3[1::2, :, 0:32] = f3[1::2, :, 0:32]
        xa3[1::2, :, 32] = t3[1::2]
        xa3[1::2, :, 33:KX] = f3[1::2, :, 32:D]
        xt = np.ascontiguousarray(
            xa3.reshape(NG, GT, KX).transpose(0, 2, 1)).astype(BF16)

        wg = np.zeros((NG, 128, WG), np.float32)
        tb4 = tile_bins.reshape(NG, 4)
        for g in range(NG):
            for pi in range(2):
                qe, qo = tb4[g, 2 * pi], tb4[g, 2 * pi + 1]
                wc = WBC * pi
                wg[g, 0:H, wc:wc + WBC] = WQ[qe]
                wg[g, H:128, wc:wc + WBC] = WQ[qo]
                wg[g, 0:H, BIAS0 + 2 * pi] = HB1[qe]
                wg[g, H:128, BIAS0 + 2 * pi] = HB1[qo]
            for i in range(4):
                trow = 96 if i % 2 == 0 else 32
                wg[g, trow, HTB + 129 * i:HTB + 129 * (i + 1)] = TQ[tb4[g, i], 0]
        bb = np.zeros((NG, 128, 3), np.float32)
        bb[:, 0:H, 0] = HB2[tb4[:, 0]]
        bb[:, H:128, 0] = HB2[tb4[:, 1]]
        bb[:, 0:H, 1] = HB2[tb4[:, 2]]
        bb[:, H:128, 1] = HB2[tb4[:, 3]]
        for i in range(4):
            bb[:, 32 * i, 2] = hb3v[tb4[:, i]]

        in_maps.append({
            "xt": xt, "wg": wg.astype(BF16), "bb": bb,
            "w1": w1a.astype(BF16), "w2": w2a.astype(BF16), "b12": b12,
        })
    return in_maps, lidx_all, order


def postprocess(core_outs, lidx_all, order):
    """core_outs: list of per-core 'out' arrays [NT, TILE] -> full [N, 1]."""
    out_sorted = np.empty(N, np.float32)
    for c in range(NCORES):
        flat = np.asarray(core_outs[c], np.float32).reshape(RPAD)
        lidx = lidx_all[c]
        valid = lidx >= 0
        seg = np.empty(RPC, np.float32)
        seg[lidx[valid]] = flat[valid]
        out_sorted[c * RPC:(c + 1) * RPC] = seg
    out = np.empty(N, np.float32)
    out[order] = out_sorted
    return out[:, None]


def kernel(t, x, dW1, db1, dW2, db2,
           hw1, htw1, hb1, hw2, htw2, hb2, hw3, htw3, hb3):
    in_maps, lidx_all, order = make_in_maps(
        t, x, dW1, db1, dW2, db2,
        hw1, htw1, hb1, hw2, htw2, hb2, hw3, htw3, hb3)
    nc = _get_program()
    res = run_bass_kernel_spmd(nc, in_maps, list(range(NCORES)))
    global last_results
    last_results = res
    return postprocess([res.results[c]["out"] for c in range(NCORES)],
                       lidx_all, order)
